# revision 7
# baseline (speedup 1.0000x reference)
"""Trainium2 Bass kernel: 3-level db4 DWT front-end (analysis + per-band
single-band reconstructions).

Input  x : [16, 128, 8192] float32
Output   : [4, 16, 128, 8192] float32  (bands: approx, d3, d2, d1)

Sharding: depthwise per-(batch, channel) row -> flatten to 2048 independent
rows of length 8192; 256 rows per NeuronCore (8 cores), two [128, *]
partition tiles per core. No cross-core communication.

Engine balance (per tile):
  - DVE: a1 + d1 (interleaved STT chains) + a2.
  - PE (f32r diag matmuls, PSUM accumulate): d2 analysis via stride-2 rhs
    slices; all synthesis. The two trailing REC_LO stages of bands 0/1 and
    the hi+lo pair of band 2 are fused into single up-4 composites (22
    taps over 4 phases: fewer PE cycles + fewer PSUM evacuations).
  - Pool: d3 + a3 chains as mul+add pairs.
  - Act: first taps are folded into the STT chains; Act does all PSUM
    evacuations (contiguous or interleaving strided f32 writes).
"""

import numpy as np

import concourse.bass as bass
import concourse.tile as tile
from concourse import bacc, mybir
from concourse.bass_utils import run_bass_kernel_spmd

F32 = mybir.dt.float32
F32R = mybir.dt.float32r
MULT = mybir.AluOpType.mult
ADD = mybir.AluOpType.add
EQ = mybir.AluOpType.is_equal

DEC_LO = np.array([-0.0105974018, 0.0328830117, 0.0308413818, -0.1870348117,
                   -0.0279837694, 0.6308807679, 0.7148465706, 0.2303778133], np.float64)
DEC_HI = np.array([-0.2303778133, 0.7148465706, -0.6308807679, -0.0279837694,
                   0.1870348117, 0.0308413818, -0.0328830117, -0.0105974018], np.float64)
REC_LO = DEC_LO[::-1].copy()
REC_HI = DEC_HI[::-1].copy()

L0, L1, L2, L3 = 8192, 4100, 2054, 1031
N_CORES = 8
ROWS_PER_CORE = 256
TILES_PER_CORE = 2
PSUM_CHUNK = 512
H1 = 1026  # even phase width for the L3 -> L2 synthesis stages (f32r psum);
           # the odd leftover column of each phase is computed on DVE


def _synth_phase_taps(w, phase):
    """(src_offset, weight) pairs for one conv_transpose phase after crop 7."""
    if phase == 0:
        return [(b, w[7 - 2 * b]) for b in range(4)]
    return [(c, w[8 - 2 * c]) for c in range(1, 5)]


def _compose_u4(w1, w2):
    """Taps of S_{w2}(S_{w1}(src)) as an up-4 map: out[4k+r] = sum w*src[k+off]."""
    out = {r: {} for r in range(4)}
    for r in range(4):
        p2 = r & 1
        c = (r - p2) // 2
        for off2, w2v in _synth_phase_taps(w2, p2):
            t = c + off2
            p1 = t & 1
            q = (t - p1) // 2
            for off1, w1v in _synth_phase_taps(w1, p1):
                out[r][q + off1] = out[r].get(q + off1, 0.0) + w2v * w1v
    return {r: sorted(out[r].items()) for r in out}


TAPS_LL = _compose_u4(REC_LO, REC_LO)   # bands 0/1 trailing lo-lo stages
TAPS_HL = _compose_u4(REC_HI, REC_LO)   # band 2 hi-then-lo


class Ctx:
    def __init__(self, nc, pool, obpool, pspool):
        self.nc = nc
        self.pool = pool
        self.obpool = obpool
        self.pspool = pspool
        self.diag = {}

    def build_consts(self):
        nc = self.nc
        ones = self.pool.tile([128, 128], F32, tag="ones")
        nc.vector.memset(ones[:], 1.0)
        ident = self.pool.tile([128, 128], F32, tag="ident")
        nc.gpsimd.affine_select(ident[:], ones[:], [[1, 128]], EQ, 0.0,
                                base=0, channel_multiplier=-1)
        vals = set()
        for w in (REC_LO, REC_HI, DEC_HI):
            for p in (0, 1):
                vals.update(float(v) for _, v in _synth_phase_taps(w, p))
        vals.update(float(v) for v in DEC_HI)
        for taps in (TAPS_LL, TAPS_HL):
            for r in range(4):
                vals.update(float(v) for _, v in taps[r])
        for i, w in enumerate(sorted(vals)):
            d = self.pool.tile([128, 128], F32R, tag=f"diag{i}")
            nc.vector.tensor_scalar_mul(d[:], ident[:], float(w))
            self.diag[float(w)] = d


def _interleave(*op_lists):
    n = max(len(l) for l in op_lists)
    for i in range(n):
        for l in op_lists:
            if i < len(l):
                l[i]()


def _ana_thunks(ctx, xp, out, w, No):
    """out[:, i] = sum_k w[k] * xp[:, 2i+k] via ACT first tap + DVE STT."""
    nc = ctx.nc
    ops = []
    for k in range(8):
        src = xp[:, k:k + 2 * No - 1:2]
        if k == 0:
            ops.append(lambda o=out, s=src, v=float(w[0]): nc.scalar.mul(o, s, v))
        else:
            ops.append(lambda o=out, s=src, v=float(w[k]):
                       nc.vector.scalar_tensor_tensor(o, s, v, o, MULT, ADD))
    return ops


def _emit_pool_ana(ctx, xp, out, w, No):
    """8-tap analysis chain on Pool via mul+add pairs (strided src ok)."""
    nc = ctx.nc
    tmp = ctx.pool.tile([128, No], F32, tag="ptmp")
    acc = ctx.pool.tile([128, No], F32, tag="pacc")
    for k in range(8):
        src = xp[:, k:k + 2 * No - 1:2]
        if k == 0:
            nc.gpsimd.tensor_scalar_mul(acc[:, :No], src, float(w[0]))
        elif k < 7:
            nc.gpsimd.tensor_scalar_mul(tmp[:, :No], src, float(w[k]))
            nc.gpsimd.tensor_tensor(acc[:, :No], acc[:, :No], tmp[:, :No], ADD)
        else:
            nc.gpsimd.tensor_scalar_mul(tmp[:, :No], src, float(w[k]))
            nc.gpsimd.tensor_tensor(out, acc[:, :No], tmp[:, :No], ADD)


def _emit_ana_pe(ctx, xp, out, w, No):
    """Analysis on PE: 8 accumulating diag matmuls with stride-2 rhs slices."""
    nc = ctx.nc
    for c0 in range(0, No, PSUM_CHUNK):
        n = min(PSUM_CHUNK, No - c0)
        ps = ctx.pspool.tile([128, PSUM_CHUNK], F32, tag="ps")
        for k in range(8):
            rhs = xp[:, 2 * c0 + k:2 * c0 + k + 2 * n - 1:2]
            nc.tensor.matmul(ps[:, :n], ctx.diag[float(w[k])][:], rhs,
                             start=(k == 0), stop=(k == 7))
        nc.scalar.copy(out[:, c0:c0 + n], ps[:, :n])


def _emit_synth_pe(ctx, x, dest, taps_by_phase, H, stride):
    """Synthesis via diag matmuls: dest[:, r::stride] (H cols per phase) =
    sum_(off,w) w * x[:, k+off], accumulated in PSUM, Act evacuation."""
    nc = ctx.nc
    for r, taps in taps_by_phase:
        for c0 in range(0, H, PSUM_CHUNK):
            n = min(PSUM_CHUNK, H - c0)
            ps = ctx.pspool.tile([128, PSUM_CHUNK], F32, tag="ps")
            for i, (off, wv) in enumerate(taps):
                rhs = x[:, c0 + off:c0 + off + n]
                nc.tensor.matmul(ps[:, :n], ctx.diag[float(wv)][:], rhs,
                                 start=(i == 0), stop=(i == len(taps) - 1))
            s0 = r + stride * c0
            nc.scalar.copy(dest[:, s0:s0 + stride * (n - 1) + 1:stride], ps[:, :n])


def _emit_synth_band(ctx, x, taps_by_phase, H, stride, y_dst):
    """Final synthesis stage of one band, emitted in two 4096-col halves so
    the staging tile is half-band sized; each half is DMA'd as it completes."""
    nc = ctx.nc
    halfW = 4096
    for h0 in range(0, H * stride, halfW):
        ob = ctx.obpool.tile([128, halfW], F32, tag="ob")
        k_lo, k_hi = h0 // stride, (h0 + halfW) // stride
        for r, taps in taps_by_phase:
            for c0 in range(k_lo, k_hi, PSUM_CHUNK):
                n = min(PSUM_CHUNK, k_hi - c0)
                ps = ctx.pspool.tile([128, PSUM_CHUNK], F32, tag="ps")
                for i, (off, wv) in enumerate(taps):
                    rhs = x[:, c0 + off:c0 + off + n]
                    nc.tensor.matmul(ps[:, :n], ctx.diag[float(wv)][:], rhs,
                                     start=(i == 0), stop=(i == len(taps) - 1))
                s0 = r + stride * c0 - h0
                nc.scalar.copy(ob[:, s0:s0 + stride * (n - 1) + 1:stride], ps[:, :n])
        nc.sync.dma_start(y_dst[:, h0:h0 + halfW], ob[:])


def _emit_reflect(ctx, xp, L):
    nc = ctx.nc
    nc.vector.tensor_copy(xp[:, 0:7], xp[:, 14:7:-1])
    nc.vector.tensor_copy(xp[:, 7 + L:14 + L], xp[:, L + 5:L - 2:-1])


def build_nc():
    nc = bacc.Bacc("TRN2", target_bir_lowering=False, debug=False,
                   num_devices=N_CORES)
    x_ap = nc.dram_tensor("x", [ROWS_PER_CORE, L0], F32, kind="ExternalInput").ap()
    y_ap = nc.dram_tensor("y", [4, ROWS_PER_CORE, L0], F32, kind="ExternalOutput").ap()

    with tile.TileContext(nc) as tc:
        with tc.tile_pool(name="bufs", bufs=1) as pool, \
             tc.tile_pool(name="co", bufs=2) as copool, \
             tc.tile_pool(name="ob", bufs=2) as obpool, \
             tc.tile_pool(name="ps", bufs=8, space="PSUM") as pspool:
            ctx = Ctx(nc, pool, obpool, pspool)
            ctx.build_consts()

            for t in range(TILES_PER_CORE):
                rows = slice(t * 128, (t + 1) * 128)

                xp = pool.tile([128, L0 + 14], F32, tag="xp")
                # split the load so d1's chain can start after the first half
                nc.sync.dma_start(xp[:, 7:7 + 4096], x_ap[rows, 0:4096])
                nc.sync.dma_start(xp[:, 7 + 4096:7 + L0], x_ap[rows, 4096:L0])
                _emit_reflect(ctx, xp, L0)

                # level 1 on DVE: d1 and a1 interleaved STT chains
                d1 = pool.tile([128, L1], F32R, tag="d1")
                a1p = pool.tile([128, L1 + 14], F32R, tag="a1p")
                _interleave(_ana_thunks(ctx, xp, d1[:], DEC_HI, L1),
                            _ana_thunks(ctx, xp, a1p[:, 7:7 + L1], DEC_LO, L1))
                _emit_reflect(ctx, a1p, L1)

                # band 3 on PE as early as possible (needs only d1)
                _emit_synth_band(ctx, d1,
                                 [(p, _synth_phase_taps(REC_HI, p)) for p in (0, 1)],
                                 L0 // 2, 2, y_ap[3, rows, :])

                # level 2: d2 on PE (strided diag), a2 on DVE
                d2 = copool.tile([128, L2], F32R, tag="d2")
                _emit_ana_pe(ctx, a1p, d2[:], DEC_HI, L2)
                a2p = pool.tile([128, L2 + 14], F32, tag="a2p")
                _interleave(_ana_thunks(ctx, a1p, a2p[:, 7:7 + L2], DEC_LO, L2))
                _emit_reflect(ctx, a2p, L2)

                # band 2 on PE: fused hi-then-lo U4 composite from d2
                _emit_synth_band(ctx, d2, list(TAPS_HL.items()), L0 // 4, 4,
                                 y_ap[2, rows, :])

                # level 3 on Pool: d3 and a3 via mul+add pairs
                d3 = pool.tile([128, L3], F32R, tag="d3")
                a3 = pool.tile([128, L3], F32R, tag="a3")
                _emit_pool_ana(ctx, a2p, d3[:], DEC_HI, L3)
                _emit_pool_ana(ctx, a2p, a3[:], DEC_LO, L3)

                def _s1_tail(src, dest, w):
                    # last output column of each phase (i = H1) on DVE
                    for p in (0, 1):
                        dcol = dest[:, 2 * H1 + p:2 * H1 + p + 1]
                        for i, (off, wv) in enumerate(_synth_phase_taps(w, p)):
                            scol = src[:, H1 + off:H1 + off + 1]
                            if i == 0:
                                nc.scalar.mul(dcol, scol, float(wv))
                            else:
                                nc.vector.scalar_tensor_tensor(
                                    dcol, scol, float(wv), dcol, MULT, ADD)

                # band 1: s1 (REC_HI) from d3 on PE -> v, then fused lo-lo U4
                v = copool.tile([128, L2], F32R, tag="v")
                _emit_synth_pe(ctx, d3, v,
                               [(p, _synth_phase_taps(REC_HI, p)) for p in (0, 1)],
                               H1, 2)
                _s1_tail(d3, v, REC_HI)
                _emit_synth_band(ctx, v, list(TAPS_LL.items()), L0 // 4, 4,
                                 y_ap[1, rows, :])

                # band 0: s1 (REC_LO) from a3 on PE -> u, then fused lo-lo U4
                u = copool.tile([128, L2], F32R, tag="u")
                _emit_synth_pe(ctx, a3, u,
                               [(p, _synth_phase_taps(REC_LO, p)) for p in (0, 1)],
                               H1, 2)
                _s1_tail(a3, u, REC_LO)
                _emit_synth_band(ctx, u, list(TAPS_LL.items()), L0 // 4, 4,
                                 y_ap[0, rows, :])

    nc.compile()
    return nc


_NC = None


def _get_nc():
    global _NC
    if _NC is None:
        _NC = build_nc()
    return _NC


def shard_inputs(x):
    rows = np.ascontiguousarray(x.reshape(-1, L0)).astype(np.float32)
    return [{"x": rows[c * ROWS_PER_CORE:(c + 1) * ROWS_PER_CORE]}
            for c in range(N_CORES)]


def unshard_outputs(results):
    out = np.empty((4, N_CORES * ROWS_PER_CORE, L0), np.float32)
    for c, r in enumerate(results):
        out[:, c * ROWS_PER_CORE:(c + 1) * ROWS_PER_CORE, :] = r["y"].astype(np.float32)
    return out.reshape(4, 16, 128, L0)


def kernel(x):
    x = np.asarray(x, np.float32)
    assert x.shape == (16, 128, L0), x.shape
    nc = _get_nc()
    res = run_bass_kernel_spmd(nc, shard_inputs(x), core_ids=list(range(N_CORES)))
    return unshard_outputs(res.results)
